# revision 6
# baseline (speedup 1.0000x reference)
"""Trainium2 Bass kernel for nn_MultiHeadAttention_53463752900838.

Math (per batch element b, one NeuronCore each — pure data parallel over B=8):
  qkv = w_qkv @ x + b_qkv                     (3072, T)
  q,k,v per head h: (64, T);  q scaled by 1/8 (folded into weights on host)
  scores[t,h,g] = sum_d q[h,d,t] k[g,d,t]     per-timestep 16x16 Gram matrix
  attn = softmax over t  (per (h,g) pair)
  context[h,d,t] = sum_g attn[t,h,g] v[g,d,t]
  out = w_out @ context + b_out               (1024, T)

Kernel layout strategy (all bf16 matmuls, fp32 PSUM accumulation):
  Pass 1 (per 256-t span): project QKV in natural (o, t) orientation,
    marshal per-head blocks into
      QT (64d, (h,t)) / KT (64d, (g,t)) / VT (16g, (d,t))
    via SBUF->SBUF DMA; per-t 16x16 scores matmuls (lhsT=KT slice, rhs=QT
    slice); fused exp during PSUM evac on ScalarE; running Z sums.
    exp(S) and VT spill to DRAM.
  Pass 2 (per span): reload, normalize by 1/Z, per-t context matmuls with
    tile_position column tiling, re-marshal context to channel-major via
    SBUF->SBUF DMA, final projection as out^T (t, o), host transposes back.
"""

import os
import sys
import contextlib

import numpy as np
import ml_dtypes

for p in ("/opt/trn_rl_repo",):
    if p not in sys.path and os.path.isdir(p):
        sys.path.insert(0, p)

import concourse.bass as bass
import concourse.tile as tile
from concourse import mybir
from concourse.bass_utils import run_bass_kernel_spmd

F32 = mybir.dt.float32
BF16 = mybir.dt.bfloat16

N_CORES = 8
C = 1024
H = 16
DK = 64
OC3 = 3072


def _split_sync_waits(nc, limit=1):
    """walrus codegen rejects >1 semaphore wait per instruction; hoist
    overflow waits onto NoOps inserted before the offending instruction."""
    counter = [0]
    n_split = 0
    for fn in nc.m.functions:
        for bb in fn.blocks:
            out = []
            for ins in bb.instructions:
                si = getattr(ins, "sync_info", None)
                waits = list(si.on_wait) if (si is not None and si.on_wait) else []
                if len(waits) > limit:
                    n_split += 1
                    extra, keep = waits[:-limit], waits[-limit:]
                    for i in range(0, len(extra), limit):
                        counter[0] += 1
                        out.append(
                            mybir.InstNoOp(
                                name=f"I-wsplit-{counter[0]}",
                                opcode="NoOp",
                                engine=ins.engine,
                                ins=[],
                                outs=[],
                                sync_info=mybir.SyncInfo(
                                    on_wait=list(extra[i : i + limit]), on_update=[]
                                ),
                            )
                        )
                    si.on_wait = keep
                out.append(ins)
            bb.instructions[:] = out
    return n_split


def build_kernel(T=4096, SPAN=256):
    NSPAN = T // SPAN
    nc = bass.Bass("TRN2", target_bir_lowering=False, debug=False)

    x_in = nc.dram_tensor("x", [C, T], BF16, kind="ExternalInput").ap()
    wq_in = nc.dram_tensor("wqT", [C, OC3], BF16, kind="ExternalInput").ap()
    bq_in = nc.dram_tensor("bqT", [1, OC3], BF16, kind="ExternalInput").ap()
    wo_in = nc.dram_tensor("woT", [C, C], BF16, kind="ExternalInput").ap()
    bo_in = nc.dram_tensor("boT", [1, C], BF16, kind="ExternalInput").ap()
    out_t = nc.dram_tensor("outT", [T, C], F32, kind="ExternalOutput").ap()
    # DRAM scratch: exp(scores) (g, (h,t)) and VT (g, (d,t)) per span
    se_d = nc.dram_tensor("se_d", [16, H * T], BF16).ap()
    vt_d = nc.dram_tensor("vt_d", [16, DK * T], BF16).ap()

    Exp = mybir.ActivationFunctionType.Exp
    Copy = mybir.ActivationFunctionType.Copy
    ADD = mybir.AluOpType.add
    MUL = mybir.AluOpType.mult

    with tile.TileContext(nc) as tc, contextlib.ExitStack() as octx:
        const = octx.enter_context(tc.tile_pool(name="const", bufs=1))
        wo_sb = []
        for k in range(8):
            w = const.tile([128, C], BF16, tag=f"wo{k}")
            nc.sync.dma_start(w[:], wo_in[k * 128 : (k + 1) * 128, :])
            wo_sb.append(w)
        bo_sb = const.tile([1, C], BF16, tag="bo")
        nc.sync.dma_start(bo_sb[:], bo_in)
        ones_t = const.tile([1, SPAN], BF16, tag="ones_t")
        nc.gpsimd.memset(ones_t[:], 1.0)
        ones128 = const.tile([1, 128], BF16, tag="ones128")
        nc.gpsimd.memset(ones128[:], 1.0)
        zacc = const.tile([16, 16], F32, tag="zacc")
        rrec = const.tile([16, 16], F32, tag="rrec")

        # ---------------- PASS 1 ----------------
        with contextlib.ExitStack() as ctx:
            wpool = ctx.enter_context(tc.tile_pool(name="wq", bufs=1))
            wq_sb = []
            for k in range(8):
                w = wpool.tile([128, OC3], BF16, tag=f"wq{k}")
                nc.sync.dma_start(w[:], wq_in[k * 128 : (k + 1) * 128, :])
                wq_sb.append(w)
            bq_sb = wpool.tile([1, OC3], BF16, tag="bq")
            nc.sync.dma_start(bq_sb[:], bq_in)

            xpool = ctx.enter_context(tc.tile_pool(name="x", bufs=2))
            stpool = ctx.enter_context(tc.tile_pool(name="stage", bufs=2))
            qkpool = ctx.enter_context(tc.tile_pool(name="qkt", bufs=2))
            vtpool = ctx.enter_context(tc.tile_pool(name="vt", bufs=1))
            sepool = ctx.enter_context(tc.tile_pool(name="se", bufs=2))
            zpool = ctx.enter_context(tc.tile_pool(name="zp", bufs=2))
            ps_a = ctx.enter_context(tc.tile_pool(name="psA", bufs=3, space="PSUM"))
            ps_s = ctx.enter_context(tc.tile_pool(name="psS", bufs=2, space="PSUM"))

            for s in range(NSPAN):
                t0 = s * SPAN
                xs = []
                for k in range(8):
                    xk = xpool.tile([128, SPAN], BF16, tag=f"x{k}")
                    nc.sync.dma_start(xk[:], x_in[k * 128 : (k + 1) * 128, t0 : t0 + SPAN])
                    xs.append(xk)

                qt = qkpool.tile([64, H * SPAN], BF16, tag="qt")
                kt = qkpool.tile([64, H * SPAN], BF16, tag="kt")
                vt = vtpool.tile([16, DK * SPAN], BF16, tag="vt")

                stages = {}
                for kind in range(3):  # 0=q, 1=k, 2=v
                    stages[kind] = stpool.tile(
                        [128, 8 * SPAN], BF16, tag=f"st{kind}", name=f"st{kind}"
                    )
                for m in range(24):
                    kind, mm = divmod(m, 8)
                    ps = ps_a.tile([128, SPAN], F32, tag="psA")
                    nc.tensor.matmul(
                        ps[:],
                        lhsT=bq_sb[0:1, m * 128 : (m + 1) * 128],
                        rhs=ones_t[:],
                        start=True,
                        stop=False,
                    )
                    for k in range(8):
                        nc.tensor.matmul(
                            ps[:],
                            lhsT=wq_sb[k][:, m * 128 : (m + 1) * 128],
                            rhs=xs[k][:],
                            start=False,
                            stop=(k == 7),
                        )
                    stg = stages[kind][:, mm * SPAN : (mm + 1) * SPAN]
                    if m % 2 == 0:
                        nc.vector.tensor_copy(stg, ps[:])
                    else:
                        nc.scalar.activation(stg, ps[:], Copy)
                # marshal: Q/K via 2 strided HWDGE copies each; V via 8 SWDGE
                for dst, kind in ((qt, 0), (kt, 1)):
                    src = stages[kind]
                    for par in range(2):
                        nc.sync.dma_start(
                            dst[0:64, :].rearrange(
                                "p (m par t) -> p m par t", m=8, par=2
                            )[:, :, par, :],
                            src[par * 64 : (par + 1) * 64, :]
                            .rearrange("p (m t) -> p m t", m=8),
                        )
                for mm in range(8):
                    nc.gpsimd.dma_start(
                        vt[2 * mm : 2 * mm + 2, :].rearrange("p (d t) -> p d t", d=DK),
                        stages[2][:, mm * SPAN : (mm + 1) * SPAN],
                    )

                qtv = qt[:].rearrange("p (h t) -> p t h", h=H)
                ktv = kt[:].rearrange("p (g t) -> p t g", g=H)
                se = sepool.tile([16, H * SPAN], BF16, tag="se")
                sev = se[:].rearrange("p (h t) -> p t h", h=H)
                for blk in range(SPAN // 32):
                    pss = ps_s.tile([16, 512], F32, tag="psS")
                    for s32 in range(32):
                        tl = blk * 32 + s32
                        nc.tensor.matmul(
                            pss[:, s32 * 16 : (s32 + 1) * 16],
                            lhsT=ktv[:, tl, :],
                            rhs=qtv[:, tl, :],
                            start=True,
                            stop=True,
                        )
                    nc.scalar.activation(
                        sev[:, blk * 32 : (blk + 1) * 32, :],
                        pss[:].rearrange("p (t h) -> p t h", h=H),
                        Exp,
                    )
                zp = zpool.tile([16, 16], F32, tag="zp")
                nc.vector.tensor_reduce(
                    zp[:],
                    se[:].rearrange("p (h t) -> p h t", h=H),
                    axis=mybir.AxisListType.X,
                    op=ADD,
                )
                if s == 0:
                    nc.vector.tensor_copy(zacc[:], zp[:])
                else:
                    nc.vector.tensor_tensor(out=zacc[:], in0=zacc[:], in1=zp[:], op=ADD)
                nc.sync.dma_start(se_d[:, s * H * SPAN : (s + 1) * H * SPAN], se[:])
                nc.sync.dma_start(vt_d[:, s * DK * SPAN : (s + 1) * DK * SPAN], vt[:])

            nc.vector.reciprocal(rrec[:], zacc[:])

        # ---------------- PASS 2 ----------------
        with contextlib.ExitStack() as ctx:
            sepool = ctx.enter_context(tc.tile_pool(name="se2", bufs=2))
            vtpool = ctx.enter_context(tc.tile_pool(name="vt2", bufs=2))
            apool = ctx.enter_context(tc.tile_pool(name="attn", bufs=2))
            cpool = ctx.enter_context(tc.tile_pool(name="csb", bufs=2))
            cnpool = ctx.enter_context(tc.tile_pool(name="cnat", bufs=2))
            opool = ctx.enter_context(tc.tile_pool(name="osb", bufs=2))
            ps_c = ctx.enter_context(tc.tile_pool(name="psC", bufs=3, space="PSUM"))
            ps_o = ctx.enter_context(tc.tile_pool(name="psO", bufs=2, space="PSUM"))

            rbc = rrec[:].unsqueeze(2).broadcast_to([16, 16, SPAN])

            for s in range(NSPAN):
                t0 = s * SPAN
                se = sepool.tile([16, H * SPAN], BF16, tag="se2")
                nc.sync.dma_start(se[:], se_d[:, s * H * SPAN : (s + 1) * H * SPAN])
                vt = vtpool.tile([16, DK * SPAN], BF16, tag="vt2")
                nc.sync.dma_start(vt[:], vt_d[:, s * DK * SPAN : (s + 1) * DK * SPAN])

                at = apool.tile([16, H * SPAN], BF16, tag="attn")
                nc.vector.tensor_tensor(
                    out=at[:].rearrange("p (h t) -> p h t", h=H),
                    in0=se[:].rearrange("p (h t) -> p h t", h=H),
                    in1=rbc,
                    op=MUL,
                )

                atv = at[:].rearrange("p (h t) -> p t h", h=H)
                vtv = vt[:].rearrange("p (d t) -> p t d", d=DK)
                # C_sb[32j+h, d*64 + tj],  t_local = j*64 + tj, tj = q*8 + s8
                csb = cpool.tile([128, DK * 64], BF16, tag="csb")
                for q in range(8):
                    psc = ps_c.tile([128, 512], F32, tag="psC")
                    for j in range(4):
                        for s8 in range(8):
                            tl = j * 64 + q * 8 + s8
                            nc.tensor.matmul(
                                psc[32 * j : 32 * j + 16, s8 * 64 : (s8 + 1) * 64],
                                lhsT=atv[:, tl, :],
                                rhs=vtv[:, tl, :],
                                start=True,
                                stop=True,
                                tile_position=(0, 32 * j),
                            )
                    nc.vector.tensor_copy(
                        csb[:]
                        .rearrange("p (d tj) -> p tj d", d=DK)[:, q * 8 : (q + 1) * 8, :],
                        psc[:].rearrange("p (s d) -> p s d", s=8),
                    )

                # marshal C_sb -> Cnat (128=(h%2)*64+d rows, free = k-block*SPAN + t)
                cnat = cnpool.tile([128, 8 * SPAN], BF16, tag="cnat")
                for j in range(4):
                    for k in range(8):
                        nc.gpsimd.dma_start(
                            cnat[:, k * SPAN + j * 64 : k * SPAN + (j + 1) * 64],
                            csb[32 * j + 2 * k : 32 * j + 2 * k + 2, :].rearrange(
                                "p (d t) -> p d t", d=DK
                            ),
                        )

                for mt in range(SPAN // 128):
                    for n in range(2):
                        pso = ps_o.tile([128, 512], F32, tag="psO")
                        nc.tensor.matmul(
                            pso[:],
                            lhsT=ones128[:],
                            rhs=bo_sb[0:1, n * 512 : (n + 1) * 512],
                            start=True,
                            stop=False,
                        )
                        for k in range(8):
                            nc.tensor.matmul(
                                pso[:],
                                lhsT=cnat[:, k * SPAN + mt * 128 : k * SPAN + mt * 128 + 128],
                                rhs=wo_sb[k][:, n * 512 : (n + 1) * 512],
                                start=False,
                                stop=(k == 7),
                            )
                        osb = opool.tile([128, 512], F32, tag="osb")
                        nc.scalar.activation(osb[:], pso[:], Copy)
                        nc.sync.dma_start(
                            out_t[t0 + mt * 128 : t0 + mt * 128 + 128, n * 512 : (n + 1) * 512],
                            osb[:],
                        )

    _split_sync_waits(nc, limit=1)
    return nc


_NC_CACHE = {}


def _get_nc(T, SPAN):
    key = (T, SPAN)
    if key not in _NC_CACHE:
        _NC_CACHE[key] = build_kernel(T, SPAN)
    return _NC_CACHE[key]


def _prep_weights(w_qkv, b_qkv, w_out, b_out):
    bf = ml_dtypes.bfloat16
    w3 = w_qkv.reshape(H, 192, C).astype(np.float32)
    qw = (w3[:, :DK, :] / 8.0).reshape(H * DK, C)
    kw = w3[:, DK : 2 * DK, :].reshape(H * DK, C)
    vw = w3[:, 2 * DK :, :].reshape(H * DK, C)
    wqT = np.concatenate([qw, kw, vw], axis=0).T.copy().astype(bf)  # (C, 3072)
    b3 = b_qkv.reshape(H, 192).astype(np.float32)
    bq = np.concatenate(
        [(b3[:, :DK] / 8.0).reshape(-1), b3[:, DK : 2 * DK].reshape(-1), b3[:, 2 * DK :].reshape(-1)]
    ).reshape(1, OC3).astype(bf)
    woT = w_out.T.copy().astype(bf)  # (C, C) rows = (h,d) h-major
    boT = b_out.reshape(1, C).astype(bf)
    return wqT, bq, woT, boT


def kernel(x, w_qkv, b_qkv, w_out, b_out, _trace=False, _span=256):
    B, _, T = x.shape
    assert B == N_CORES
    nc = _get_nc(T, _span)
    wqT, bq, woT, boT = _prep_weights(w_qkv, b_qkv, w_out, b_out)
    bf = ml_dtypes.bfloat16
    in_maps = []
    for b in range(B):
        in_maps.append(
            {
                "x": x[b].astype(bf),
                "wqT": wqT,
                "bqT": bq,
                "woT": woT,
                "boT": boT,
            }
        )
    res = run_bass_kernel_spmd(nc, in_maps, list(range(N_CORES)), trace=_trace)
    out = np.stack([res.results[b]["outT"].T for b in range(B)], axis=0)
    if _trace:
        kernel.last_exec_time_ns = res.exec_time_ns
        kernel.last_results = res
    return out.astype(np.float32)


# revision 12
# speedup vs baseline: 1.2017x; 1.2017x over previous
"""Trainium2 Bass kernel for nn_MultiHeadAttention_53463752900838.

Math (per batch element b, one NeuronCore each — pure data parallel over B=8):
  qkv = w_qkv @ x + b_qkv                     (3072, T)
  q,k,v per head h: (64, T);  q scaled by 1/8 (folded into weights on host)
  scores[t,h,g] = sum_d q[h,d,t] k[g,d,t]     per-timestep 16x16 Gram matrix
  attn = softmax over t  (per (h,g) pair)
  context[h,d,t] = sum_g attn[t,h,g] v[g,d,t]
  out = w_out @ context + b_out               (1024, T)

Kernel layout strategy (all bf16 matmuls, fp32 PSUM accumulation):
  Pass 1 (per 256-t span): project QKV in natural (o, t) orientation,
    marshal per-head blocks into
      QT (64d, (h,t)) / KT (64d, (g,t)) / VT (16g, (d,t))
    via SBUF->SBUF DMA; per-t 16x16 scores matmuls (lhsT=KT slice, rhs=QT
    slice); fused exp during PSUM evac on ScalarE; running Z sums.
    exp(S) and VT spill to DRAM.
  Pass 2 (per span): reload, normalize by 1/Z, per-t context matmuls with
    tile_position column tiling, re-marshal context to channel-major via
    SBUF->SBUF DMA, final projection as out^T (t, o), host transposes back.
"""

import os
import sys
import contextlib

import numpy as np
import ml_dtypes

for p in ("/opt/trn_rl_repo",):
    if p not in sys.path and os.path.isdir(p):
        sys.path.insert(0, p)

import concourse.bass as bass
import concourse.tile as tile
from concourse import mybir
from concourse.bass_utils import run_bass_kernel_spmd

F32 = mybir.dt.float32
BF16 = mybir.dt.bfloat16

N_CORES = 8
C = 1024
H = 16
DK = 64
OC3 = 3072


_WAITS2_OK = {
    "InstMatmult",
    "InstLdweights",
    "InstTensorCopy",
    "InstActivation",
    "InstTensorTensor",
    "InstTensorReduce",
    "InstDMACopy",
    "InstTensorScalarPtr",
    "InstMemset",
}


def _split_sync_waits(nc, limit=1):
    """walrus codegen rejects too many semaphore waits per instruction (CTRL
    class takes 1); hoist overflow waits onto NoOps inserted before the
    offending instruction. Compute/DMA instructions take 2."""
    counter = [0]
    n_split = 0
    for fn in nc.m.functions:
        for bb in fn.blocks:
            out = []
            for ins in bb.instructions:
                si = getattr(ins, "sync_info", None)
                waits = list(si.on_wait) if (si is not None and si.on_wait) else []
                if len(waits) > limit:
                    n_split += 1
                    extra, keep = waits[:-limit], waits[-limit:]
                    for i in range(0, len(extra), limit):
                        counter[0] += 1
                        out.append(
                            mybir.InstNoOp(
                                name=f"I-wsplit-{counter[0]}",
                                opcode="NoOp",
                                engine=ins.engine,
                                ins=[],
                                outs=[],
                                sync_info=mybir.SyncInfo(
                                    on_wait=list(extra[i : i + limit]), on_update=[]
                                ),
                            )
                        )
                    si.on_wait = keep
                out.append(ins)
            bb.instructions[:] = out
    return n_split


def build_kernel(T=4096, SPAN=256):
    NSPAN = T // SPAN
    nc = bass.Bass("TRN2", target_bir_lowering=False, debug=False)

    x_in = nc.dram_tensor("x", [C, T], BF16, kind="ExternalInput").ap()
    wq_in = nc.dram_tensor("wqT", [C, OC3], BF16, kind="ExternalInput").ap()
    bq_in = nc.dram_tensor("bqT", [1, OC3], BF16, kind="ExternalInput").ap()
    wo_in = nc.dram_tensor("woT", [C, C], BF16, kind="ExternalInput").ap()
    bo_in = nc.dram_tensor("boT", [1, C], BF16, kind="ExternalInput").ap()
    out_t = nc.dram_tensor("outT", [T, C], F32, kind="ExternalOutput").ap()
    # DRAM scratch: exp(scores) (g, (h,t)) and VT (g, (d,t)) per span
    se_d = nc.dram_tensor("se_d", [16, H * T], BF16).ap()
    vt_d = nc.dram_tensor("vt_d", [16, DK * T], BF16).ap()

    Exp = mybir.ActivationFunctionType.Exp
    Copy = mybir.ActivationFunctionType.Copy
    ADD = mybir.AluOpType.add
    MUL = mybir.AluOpType.mult

    with tile.TileContext(nc) as tc, contextlib.ExitStack() as octx:
        const = octx.enter_context(tc.tile_pool(name="const", bufs=1))
        wo_sb = []
        for k in range(8):
            w = const.tile([128, C], BF16, tag=f"wo{k}")
            nc.sync.dma_start(w[:], wo_in[k * 128 : (k + 1) * 128, :])
            wo_sb.append(w)
        bo_sb = const.tile([1, C], BF16, tag="bo")
        nc.sync.dma_start(bo_sb[:], bo_in)
        ones_t = const.tile([1, SPAN], BF16, tag="ones_t")
        nc.gpsimd.memset(ones_t[:], 1.0)
        ones128 = const.tile([1, 128], BF16, tag="ones128")
        nc.gpsimd.memset(ones128[:], 1.0)
        zacc = const.tile([16, 16], F32, tag="zacc")
        rrec = const.tile([16, 16], F32, tag="rrec")

        # ---------------- PASS 1 ----------------
        with contextlib.ExitStack() as ctx:
            wpool = ctx.enter_context(tc.tile_pool(name="wq", bufs=1))
            wq_sb = []
            for k in range(8):
                w = wpool.tile([128, OC3], BF16, tag=f"wq{k}")
                nc.sync.dma_start(w[:], wq_in[k * 128 : (k + 1) * 128, :])
                wq_sb.append(w)
            bq_sb = wpool.tile([1, OC3], BF16, tag="bq")
            nc.sync.dma_start(bq_sb[:], bq_in)

            xpool = ctx.enter_context(tc.tile_pool(name="x", bufs=2))
            stpool = ctx.enter_context(tc.tile_pool(name="stage", bufs=2))
            qkpool = ctx.enter_context(tc.tile_pool(name="qkt", bufs=2))
            vtpool = ctx.enter_context(tc.tile_pool(name="vt", bufs=1))
            sepool = ctx.enter_context(tc.tile_pool(name="se", bufs=2))
            zpool = ctx.enter_context(tc.tile_pool(name="zp", bufs=2))
            ps_a = ctx.enter_context(tc.tile_pool(name="psA", bufs=3, space="PSUM"))
            ps_s = ctx.enter_context(tc.tile_pool(name="psS", bufs=2, space="PSUM"))

            for s in range(NSPAN):
                t0 = s * SPAN
                xs = []
                for k in range(8):
                    xk = xpool.tile([128, SPAN], BF16, tag=f"x{k}")
                    nc.sync.dma_start(xk[:], x_in[k * 128 : (k + 1) * 128, t0 : t0 + SPAN])
                    xs.append(xk)

                qt = qkpool.tile([64, H * SPAN], BF16, tag="qt")
                kt = qkpool.tile([64, H * SPAN], BF16, tag="kt")
                vt = vtpool.tile([16, DK * SPAN], BF16, tag="vt")

                stages = {}
                for kind in range(3):  # 0=q, 1=k, 2=v
                    stages[kind] = stpool.tile(
                        [128, 8 * SPAN], BF16, tag=f"st{kind}", name=f"st{kind}"
                    )
                for m in range(24):
                    kind, mm = divmod(m, 8)
                    ps = ps_a.tile([128, SPAN], F32, tag="psA")
                    nc.tensor.matmul(
                        ps[:],
                        lhsT=bq_sb[0:1, m * 128 : (m + 1) * 128],
                        rhs=ones_t[:],
                        start=True,
                        stop=False,
                    )
                    for k in range(8):
                        nc.tensor.matmul(
                            ps[:],
                            lhsT=wq_sb[k][:, m * 128 : (m + 1) * 128],
                            rhs=xs[k][:],
                            start=False,
                            stop=(k == 7),
                        )
                    stg = stages[kind][:, mm * SPAN : (mm + 1) * SPAN]
                    if m % 2 == 0:
                        nc.vector.tensor_copy(stg, ps[:])
                    else:
                        nc.scalar.activation(stg, ps[:], Copy)
                # marshal: Q/K via 2 strided HWDGE copies each; V via 8 SWDGE
                for dst, kind in ((qt, 0), (kt, 1)):
                    src = stages[kind]
                    for par in range(2):
                        nc.sync.dma_start(
                            dst[0:64, :].rearrange(
                                "p (m par t) -> p m par t", m=8, par=2
                            )[:, :, par, :],
                            src[par * 64 : (par + 1) * 64, :]
                            .rearrange("p (m t) -> p m t", m=8),
                        )
                for mm in range(8):
                    nc.gpsimd.dma_start(
                        vt[2 * mm : 2 * mm + 2, :].rearrange("p (d t) -> p d t", d=DK),
                        stages[2][:, mm * SPAN : (mm + 1) * SPAN],
                    )

                qtv = qt[:].rearrange("p (h t) -> p t h", h=H)
                ktv = kt[:].rearrange("p (g t) -> p t g", g=H)
                se = sepool.tile([16, H * SPAN], BF16, tag="se")
                sev = se[:].rearrange("p (h t) -> p t h", h=H)
                for blk in range(SPAN // 32):
                    pss = ps_s.tile([16, 512], F32, tag="psS")
                    for s32 in range(32):
                        tl = blk * 32 + s32
                        nc.tensor.matmul(
                            pss[:, s32 * 16 : (s32 + 1) * 16],
                            lhsT=ktv[:, tl, :],
                            rhs=qtv[:, tl, :],
                            start=True,
                            stop=True,
                        )
                    nc.scalar.activation(
                        sev[:, blk * 32 : (blk + 1) * 32, :],
                        pss[:].rearrange("p (t h) -> p t h", h=H),
                        Exp,
                    )
                zp = zpool.tile([16, 16], F32, tag="zp")
                nc.vector.tensor_reduce(
                    zp[:],
                    se[:].rearrange("p (h t) -> p h t", h=H),
                    axis=mybir.AxisListType.X,
                    op=ADD,
                )
                if s == 0:
                    nc.vector.tensor_copy(zacc[:], zp[:])
                else:
                    nc.vector.tensor_tensor(out=zacc[:], in0=zacc[:], in1=zp[:], op=ADD)
                nc.sync.dma_start(se_d[:, s * H * SPAN : (s + 1) * H * SPAN], se[:])
                nc.sync.dma_start(vt_d[:, s * DK * SPAN : (s + 1) * DK * SPAN], vt[:])

            nc.vector.reciprocal(rrec[:], zacc[:])

        # ---------------- PASS 2 ----------------
        with contextlib.ExitStack() as ctx:
            sepool = ctx.enter_context(tc.tile_pool(name="se2", bufs=2))
            vtpool = ctx.enter_context(tc.tile_pool(name="vt2", bufs=2))
            apool = ctx.enter_context(tc.tile_pool(name="attn", bufs=2))
            cpool = ctx.enter_context(tc.tile_pool(name="csb", bufs=2))
            cnpool = ctx.enter_context(tc.tile_pool(name="cnat", bufs=1))
            opool = ctx.enter_context(tc.tile_pool(name="osb", bufs=2))
            ps_c = ctx.enter_context(tc.tile_pool(name="psC", bufs=3, space="PSUM"))
            ps_o = ctx.enter_context(tc.tile_pool(name="psO", bufs=2, space="PSUM"))

            rbc = rrec[:].unsqueeze(2).broadcast_to([16, 16, SPAN])

            NW = min(4, NSPAN)
            SPC = NW * SPAN  # context accumulation block (1024 t)
            for sb_ in range(NSPAN // NW):
                tB0 = sb_ * SPC
                # C_sb[32j+h, d*256 + w*64 + u]: t_in_block = w*SPAN + j*64 + u
                csb = cpool.tile([128, DK * 64 * NW], BF16, tag="csb")
                for w in range(NW):
                    s = sb_ * NW + w
                    se = sepool.tile([16, H * SPAN], BF16, tag="se2")
                    nc.sync.dma_start(se[:], se_d[:, s * H * SPAN : (s + 1) * H * SPAN])
                    vt = vtpool.tile([16, DK * SPAN], BF16, tag="vt2")
                    nc.sync.dma_start(vt[:], vt_d[:, s * DK * SPAN : (s + 1) * DK * SPAN])

                    at = apool.tile([16, H * SPAN], BF16, tag="attn")
                    nc.vector.tensor_tensor(
                        out=at[:].rearrange("p (h t) -> p h t", h=H),
                        in0=se[:].rearrange("p (h t) -> p h t", h=H),
                        in1=rbc,
                        op=MUL,
                    )

                    atv = at[:].rearrange("p (h t) -> p t h", h=H)
                    vtv = vt[:].rearrange("p (d t) -> p t d", d=DK)
                    for q in range(8):
                        psc = ps_c.tile([128, 512], F32, tag="psC")
                        for j in range(4):
                            for s8 in range(8):
                                tl = j * 64 + q * 8 + s8
                                nc.tensor.matmul(
                                    psc[32 * j : 32 * j + 16, s8 * 64 : (s8 + 1) * 64],
                                    lhsT=atv[:, tl, :],
                                    rhs=vtv[:, tl, :],
                                    start=True,
                                    stop=True,
                                    tile_position=(0, 32 * j),
                                )
                        nc.vector.tensor_copy(
                            csb[:].rearrange("p (d tj) -> p tj d", d=DK)[
                                :, w * 64 + q * 8 : w * 64 + (q + 1) * 8, :
                            ],
                            psc[:].rearrange("p (s d) -> p s d", s=8),
                        )

                # marshal: Cnat rows (h%2)*64+d, free = k*SPC + w*SPAN + j*64 + u
                cnat = cnpool.tile([128, 8 * SPC], BF16, tag="cnat")
                for j in range(4):
                    for k in range(8):
                        nc.gpsimd.dma_start(
                            cnat[:, :]
                            .rearrange("p (kk w j u) -> p kk w j u", kk=8, w=NW, j=4)[
                                :, k, :, j, :
                            ],
                            csb[32 * j + 2 * k : 32 * j + 2 * k + 2, :].rearrange(
                                "p (d w u) -> p d w u", d=DK, w=NW
                            ),
                        )

                for mt in range(SPC // 128):
                    for n in range(2):
                        pso = ps_o.tile([128, 512], F32, tag="psO")
                        nc.tensor.matmul(
                            pso[:],
                            lhsT=ones128[:],
                            rhs=bo_sb[0:1, n * 512 : (n + 1) * 512],
                            start=True,
                            stop=False,
                        )
                        for k in range(8):
                            nc.tensor.matmul(
                                pso[:],
                                lhsT=cnat[:, k * SPC + mt * 128 : k * SPC + mt * 128 + 128],
                                rhs=wo_sb[k][:, n * 512 : (n + 1) * 512],
                                start=False,
                                stop=(k == 7),
                            )
                        osb = opool.tile([128, 512], F32, tag="osb")
                        nc.scalar.activation(osb[:], pso[:], Copy)
                        nc.sync.dma_start(
                            out_t[tB0 + mt * 128 : tB0 + mt * 128 + 128, n * 512 : (n + 1) * 512],
                            osb[:],
                        )

    _split_sync_waits(nc, limit=1)
    return nc


_NC_CACHE = {}


def _get_nc(T, SPAN):
    key = (T, SPAN)
    if key not in _NC_CACHE:
        _NC_CACHE[key] = build_kernel(T, SPAN)
    return _NC_CACHE[key]


def _prep_weights(w_qkv, b_qkv, w_out, b_out):
    bf = ml_dtypes.bfloat16
    w3 = w_qkv.reshape(H, 192, C).astype(np.float32)
    qw = (w3[:, :DK, :] / 8.0).reshape(H * DK, C)
    kw = w3[:, DK : 2 * DK, :].reshape(H * DK, C)
    vw = w3[:, 2 * DK :, :].reshape(H * DK, C)
    wqT = np.concatenate([qw, kw, vw], axis=0).T.copy().astype(bf)  # (C, 3072)
    b3 = b_qkv.reshape(H, 192).astype(np.float32)
    bq = np.concatenate(
        [(b3[:, :DK] / 8.0).reshape(-1), b3[:, DK : 2 * DK].reshape(-1), b3[:, 2 * DK :].reshape(-1)]
    ).reshape(1, OC3).astype(bf)
    woT = w_out.T.copy().astype(bf)  # (C, C) rows = (h,d) h-major
    boT = b_out.reshape(1, C).astype(bf)
    return wqT, bq, woT, boT


def kernel(x, w_qkv, b_qkv, w_out, b_out, _trace=False, _span=256):
    B, _, T = x.shape
    assert B == N_CORES
    nc = _get_nc(T, _span)
    wqT, bq, woT, boT = _prep_weights(w_qkv, b_qkv, w_out, b_out)
    bf = ml_dtypes.bfloat16
    in_maps = []
    for b in range(B):
        in_maps.append(
            {
                "x": x[b].astype(bf),
                "wqT": wqT,
                "bqT": bq,
                "woT": woT,
                "boT": boT,
            }
        )
    res = run_bass_kernel_spmd(nc, in_maps, list(range(N_CORES)), trace=_trace)
    out = np.stack([res.results[b]["outT"].T for b in range(B)], axis=0)
    if _trace:
        kernel.last_exec_time_ns = res.exec_time_ns
        kernel.last_results = res
    return out.astype(np.float32)


# revision 13
# speedup vs baseline: 1.2101x; 1.0070x over previous
"""Trainium2 Bass kernel for nn_MultiHeadAttention_53463752900838.

Math (per batch element b, one NeuronCore each — pure data parallel over B=8):
  qkv = w_qkv @ x + b_qkv                     (3072, T)
  q,k,v per head h: (64, T);  q scaled by 1/8 (folded into weights on host)
  scores[t,h,g] = sum_d q[h,d,t] k[g,d,t]     per-timestep 16x16 Gram matrix
  attn = softmax over t  (per (h,g) pair)
  context[h,d,t] = sum_g attn[t,h,g] v[g,d,t]
  out = w_out @ context + b_out               (1024, T)

Kernel layout strategy (all bf16 matmuls, fp32 PSUM accumulation):
  Pass 1 (per 256-t span): project QKV in natural (o, t) orientation,
    marshal per-head blocks into
      QT (64d, (h,t)) / KT (64d, (g,t)) / VT (16g, (d,t))
    via SBUF->SBUF DMA; per-t 16x16 scores matmuls (lhsT=KT slice, rhs=QT
    slice); fused exp during PSUM evac on ScalarE; running Z sums.
    exp(S) and VT spill to DRAM.
  Pass 2 (per span): reload, normalize by 1/Z, per-t context matmuls with
    tile_position column tiling, re-marshal context to channel-major via
    SBUF->SBUF DMA, final projection as out^T (t, o), host transposes back.
"""

import os
import sys
import contextlib

import numpy as np
import ml_dtypes

for p in ("/opt/trn_rl_repo",):
    if p not in sys.path and os.path.isdir(p):
        sys.path.insert(0, p)

import concourse.bass as bass
import concourse.tile as tile
from concourse import mybir
from concourse.bass_utils import run_bass_kernel_spmd

F32 = mybir.dt.float32
BF16 = mybir.dt.bfloat16

N_CORES = 8
C = 1024
H = 16
DK = 64
OC3 = 3072


_WAITS2_OK = {
    "InstMatmult",
    "InstLdweights",
    "InstTensorCopy",
    "InstActivation",
    "InstTensorTensor",
    "InstTensorReduce",
    "InstDMACopy",
    "InstTensorScalarPtr",
    "InstMemset",
}


def _split_sync_waits(nc, limit=1):
    """walrus codegen rejects too many semaphore waits per instruction (CTRL
    class takes 1); hoist overflow waits onto NoOps inserted before the
    offending instruction. Compute/DMA instructions take 2."""
    counter = [0]
    n_split = 0
    for fn in nc.m.functions:
        for bb in fn.blocks:
            out = []
            for ins in bb.instructions:
                si = getattr(ins, "sync_info", None)
                waits = list(si.on_wait) if (si is not None and si.on_wait) else []
                if len(waits) > limit:
                    n_split += 1
                    extra, keep = waits[:-limit], waits[-limit:]
                    for i in range(0, len(extra), limit):
                        counter[0] += 1
                        out.append(
                            mybir.InstNoOp(
                                name=f"I-wsplit-{counter[0]}",
                                opcode="NoOp",
                                engine=ins.engine,
                                ins=[],
                                outs=[],
                                sync_info=mybir.SyncInfo(
                                    on_wait=list(extra[i : i + limit]), on_update=[]
                                ),
                            )
                        )
                    si.on_wait = keep
                out.append(ins)
            bb.instructions[:] = out
    return n_split


def build_kernel(T=4096, SPAN=256):
    NSPAN = T // SPAN
    nc = bass.Bass("TRN2", target_bir_lowering=False, debug=False)

    x_in = nc.dram_tensor("x", [C, T], BF16, kind="ExternalInput").ap()
    wq_in = nc.dram_tensor("wqT", [C, OC3], BF16, kind="ExternalInput").ap()
    bq_in = nc.dram_tensor("bqT", [1, OC3], BF16, kind="ExternalInput").ap()
    wo_in = nc.dram_tensor("woT", [C, C], BF16, kind="ExternalInput").ap()
    bo_in = nc.dram_tensor("boT", [1, C], BF16, kind="ExternalInput").ap()
    out_t = nc.dram_tensor("outT", [T, C], F32, kind="ExternalOutput").ap()
    # DRAM scratch: exp(scores) (g, (h,t)) and VT (g, (d,t)) per span
    se_d = nc.dram_tensor("se_d", [16, H * T], BF16).ap()
    vt_d = nc.dram_tensor("vt_d", [16, DK * T], BF16).ap()

    Exp = mybir.ActivationFunctionType.Exp
    Copy = mybir.ActivationFunctionType.Copy
    ADD = mybir.AluOpType.add
    MUL = mybir.AluOpType.mult

    with tile.TileContext(nc) as tc, contextlib.ExitStack() as octx:
        const = octx.enter_context(tc.tile_pool(name="const", bufs=1))
        wo_sb = []
        for k in range(8):
            w = const.tile([128, C], BF16, tag=f"wo{k}")
            nc.sync.dma_start(w[:], wo_in[k * 128 : (k + 1) * 128, :])
            wo_sb.append(w)
        bo_sb = const.tile([1, C], BF16, tag="bo")
        nc.sync.dma_start(bo_sb[:], bo_in)
        ones_t = const.tile([1, SPAN], BF16, tag="ones_t")
        nc.gpsimd.memset(ones_t[:], 1.0)
        ones128 = const.tile([1, 128], BF16, tag="ones128")
        nc.gpsimd.memset(ones128[:], 1.0)
        zacc = const.tile([16, 16], F32, tag="zacc")
        rrec = const.tile([16, 16], F32, tag="rrec")

        # ---------------- PASS 1 ----------------
        with contextlib.ExitStack() as ctx:
            wpool = ctx.enter_context(tc.tile_pool(name="wq", bufs=1))
            wq_sb = []
            for k in range(8):
                w = wpool.tile([128, OC3], BF16, tag=f"wq{k}")
                nc.sync.dma_start(w[:], wq_in[k * 128 : (k + 1) * 128, :])
                wq_sb.append(w)
            bq_sb = wpool.tile([1, OC3], BF16, tag="bq")
            nc.sync.dma_start(bq_sb[:], bq_in)

            xpool = ctx.enter_context(tc.tile_pool(name="x", bufs=2))
            stpool = ctx.enter_context(tc.tile_pool(name="stage", bufs=2))
            qkpool = ctx.enter_context(tc.tile_pool(name="qkt", bufs=2))
            vtpool = ctx.enter_context(tc.tile_pool(name="vt", bufs=1))
            sepool = ctx.enter_context(tc.tile_pool(name="se", bufs=2))
            zpool = ctx.enter_context(tc.tile_pool(name="zp", bufs=2))
            ps_a = ctx.enter_context(tc.tile_pool(name="psA", bufs=3, space="PSUM"))
            ps_s = ctx.enter_context(tc.tile_pool(name="psS", bufs=2, space="PSUM"))

            for s in range(NSPAN):
                t0 = s * SPAN
                xs = []
                for k in range(8):
                    xk = xpool.tile([128, SPAN], BF16, tag=f"x{k}")
                    nc.sync.dma_start(xk[:], x_in[k * 128 : (k + 1) * 128, t0 : t0 + SPAN])
                    xs.append(xk)

                qt = qkpool.tile([64, H * SPAN], BF16, tag="qt")
                kt = qkpool.tile([64, H * SPAN], BF16, tag="kt")
                vt = vtpool.tile([16, DK * SPAN], BF16, tag="vt")

                stages = {}
                for kind in range(3):  # 0=q, 1=k, 2=v
                    stages[kind] = stpool.tile(
                        [128, 8 * SPAN], BF16, tag=f"st{kind}", name=f"st{kind}"
                    )
                for m in range(24):
                    kind, mm = divmod(m, 8)
                    ps = ps_a.tile([128, SPAN], F32, tag="psA")
                    nc.tensor.matmul(
                        ps[:],
                        lhsT=bq_sb[0:1, m * 128 : (m + 1) * 128],
                        rhs=ones_t[:],
                        start=True,
                        stop=False,
                    )
                    for k in range(8):
                        nc.tensor.matmul(
                            ps[:],
                            lhsT=wq_sb[k][:, m * 128 : (m + 1) * 128],
                            rhs=xs[k][:],
                            start=False,
                            stop=(k == 7),
                        )
                    stg = stages[kind][:, mm * SPAN : (mm + 1) * SPAN]
                    if m % 2 == 0:
                        nc.vector.tensor_copy(stg, ps[:])
                    else:
                        nc.scalar.activation(stg, ps[:], Copy)
                # marshal: Q/K via 2 strided HWDGE copies each; V via 8 SWDGE
                for dst, kind in ((qt, 0), (kt, 1)):
                    src = stages[kind]
                    for par in range(2):
                        nc.sync.dma_start(
                            dst[0:64, :].rearrange(
                                "p (m par t) -> p m par t", m=8, par=2
                            )[:, :, par, :],
                            src[par * 64 : (par + 1) * 64, :]
                            .rearrange("p (m t) -> p m t", m=8),
                        )
                for mm in range(8):
                    nc.gpsimd.dma_start(
                        vt[2 * mm : 2 * mm + 2, :].rearrange("p (d t) -> p d t", d=DK),
                        stages[2][:, mm * SPAN : (mm + 1) * SPAN],
                    )

                qtv = qt[:].rearrange("p (h t) -> p t h", h=H)
                ktv = kt[:].rearrange("p (g t) -> p t g", g=H)
                se = sepool.tile([16, H * SPAN], BF16, tag="se")
                sev = se[:].rearrange("p (h t) -> p t h", h=H)
                for blk in range(SPAN // 32):
                    pss = ps_s.tile([16, 512], F32, tag="psS")
                    for s32 in range(32):
                        tl = blk * 32 + s32
                        nc.tensor.matmul(
                            pss[:, s32 * 16 : (s32 + 1) * 16],
                            lhsT=ktv[:, tl, :],
                            rhs=qtv[:, tl, :],
                            start=True,
                            stop=True,
                        )
                    nc.scalar.activation(
                        sev[:, blk * 32 : (blk + 1) * 32, :],
                        pss[:].rearrange("p (t h) -> p t h", h=H),
                        Exp,
                    )
                zp = zpool.tile([16, 16], F32, tag="zp")
                nc.vector.tensor_reduce(
                    zp[:],
                    se[:].rearrange("p (h t) -> p h t", h=H),
                    axis=mybir.AxisListType.X,
                    op=ADD,
                )
                if s == 0:
                    nc.vector.tensor_copy(zacc[:], zp[:])
                else:
                    nc.vector.tensor_tensor(out=zacc[:], in0=zacc[:], in1=zp[:], op=ADD)
                nc.sync.dma_start(se_d[:, s * H * SPAN : (s + 1) * H * SPAN], se[:])
                nc.sync.dma_start(vt_d[:, s * DK * SPAN : (s + 1) * DK * SPAN], vt[:])

            nc.vector.reciprocal(rrec[:], zacc[:])

        # ---------------- PASS 2 ----------------
        with contextlib.ExitStack() as ctx:
            sepool = ctx.enter_context(tc.tile_pool(name="se2", bufs=2))
            vtpool = ctx.enter_context(tc.tile_pool(name="vt2", bufs=2))
            apool = ctx.enter_context(tc.tile_pool(name="attn", bufs=2))
            cpool = ctx.enter_context(tc.tile_pool(name="csb", bufs=2))
            cnpool = ctx.enter_context(tc.tile_pool(name="cnat", bufs=1))
            opool = ctx.enter_context(tc.tile_pool(name="osb", bufs=2))
            ps_c = ctx.enter_context(tc.tile_pool(name="psC", bufs=3, space="PSUM"))
            ps_o = ctx.enter_context(tc.tile_pool(name="psO", bufs=2, space="PSUM"))

            rbc = rrec[:].unsqueeze(2).broadcast_to([16, 16, SPAN])

            NW = min(4, NSPAN)
            SPC = NW * SPAN  # context accumulation block (1024 t)
            for sb_ in range(NSPAN // NW):
                tB0 = sb_ * SPC
                # C_sb[32j+h, d*256 + w*64 + u]: t_in_block = w*SPAN + j*64 + u
                csb = cpool.tile([128, DK * 64 * NW], BF16, tag="csb")
                for w in range(NW):
                    s = sb_ * NW + w
                    se = sepool.tile([16, H * SPAN], BF16, tag="se2")
                    nc.sync.dma_start(se[:], se_d[:, s * H * SPAN : (s + 1) * H * SPAN])
                    vt = vtpool.tile([16, DK * SPAN], BF16, tag="vt2")
                    nc.sync.dma_start(vt[:], vt_d[:, s * DK * SPAN : (s + 1) * DK * SPAN])

                    at = apool.tile([16, H * SPAN], BF16, tag="attn")
                    nc.vector.tensor_tensor(
                        out=at[:].rearrange("p (h t) -> p h t", h=H),
                        in0=se[:].rearrange("p (h t) -> p h t", h=H),
                        in1=rbc,
                        op=MUL,
                    )

                    atv = at[:].rearrange("p (h t) -> p t h", h=H)
                    vtv = vt[:].rearrange("p (d t) -> p t d", d=DK)
                    for q in range(8):
                        psc = ps_c.tile([128, 512], F32, tag="psC")
                        for j in range(4):
                            for s8 in range(8):
                                tl = j * 64 + q * 8 + s8
                                nc.tensor.matmul(
                                    psc[32 * j : 32 * j + 16, s8 * 64 : (s8 + 1) * 64],
                                    lhsT=atv[:, tl, :],
                                    rhs=vtv[:, tl, :],
                                    start=True,
                                    stop=True,
                                    tile_position=(0, 32 * j),
                                )
                        csb_dst = csb[:].rearrange("p (d tj) -> p tj d", d=DK)[
                            :, w * 64 + q * 8 : w * 64 + (q + 1) * 8, :
                        ]
                        psc_src = psc[:].rearrange("p (s d) -> p s d", s=8)
                        if q % 2 == 0:
                            nc.vector.tensor_copy(csb_dst, psc_src)
                        else:
                            nc.scalar.activation(csb_dst, psc_src, Copy)

                # marshal: Cnat rows (h%2)*64+d, free = k*SPC + w*SPAN + j*64 + u
                cnat = cnpool.tile([128, 8 * SPC], BF16, tag="cnat")
                for j in range(4):
                    for k in range(8):
                        nc.gpsimd.dma_start(
                            cnat[:, :]
                            .rearrange("p (kk w j u) -> p kk w j u", kk=8, w=NW, j=4)[
                                :, k, :, j, :
                            ],
                            csb[32 * j + 2 * k : 32 * j + 2 * k + 2, :].rearrange(
                                "p (d w u) -> p d w u", d=DK, w=NW
                            ),
                        )

                for mt in range(SPC // 128):
                    for n in range(2):
                        pso = ps_o.tile([128, 512], F32, tag="psO")
                        nc.tensor.matmul(
                            pso[:],
                            lhsT=ones128[:],
                            rhs=bo_sb[0:1, n * 512 : (n + 1) * 512],
                            start=True,
                            stop=False,
                        )
                        for k in range(8):
                            nc.tensor.matmul(
                                pso[:],
                                lhsT=cnat[:, k * SPC + mt * 128 : k * SPC + mt * 128 + 128],
                                rhs=wo_sb[k][:, n * 512 : (n + 1) * 512],
                                start=False,
                                stop=(k == 7),
                            )
                        osb = opool.tile([128, 512], F32, tag="osb")
                        nc.scalar.activation(osb[:], pso[:], Copy)
                        nc.sync.dma_start(
                            out_t[tB0 + mt * 128 : tB0 + mt * 128 + 128, n * 512 : (n + 1) * 512],
                            osb[:],
                        )

    _split_sync_waits(nc, limit=1)
    return nc


_NC_CACHE = {}


def _get_nc(T, SPAN):
    key = (T, SPAN)
    if key not in _NC_CACHE:
        _NC_CACHE[key] = build_kernel(T, SPAN)
    return _NC_CACHE[key]


def _prep_weights(w_qkv, b_qkv, w_out, b_out):
    bf = ml_dtypes.bfloat16
    w3 = w_qkv.reshape(H, 192, C).astype(np.float32)
    qw = (w3[:, :DK, :] / 8.0).reshape(H * DK, C)
    kw = w3[:, DK : 2 * DK, :].reshape(H * DK, C)
    vw = w3[:, 2 * DK :, :].reshape(H * DK, C)
    wqT = np.concatenate([qw, kw, vw], axis=0).T.copy().astype(bf)  # (C, 3072)
    b3 = b_qkv.reshape(H, 192).astype(np.float32)
    bq = np.concatenate(
        [(b3[:, :DK] / 8.0).reshape(-1), b3[:, DK : 2 * DK].reshape(-1), b3[:, 2 * DK :].reshape(-1)]
    ).reshape(1, OC3).astype(bf)
    woT = w_out.T.copy().astype(bf)  # (C, C) rows = (h,d) h-major
    boT = b_out.reshape(1, C).astype(bf)
    return wqT, bq, woT, boT


def kernel(x, w_qkv, b_qkv, w_out, b_out, _trace=False, _span=256):
    B, _, T = x.shape
    assert B == N_CORES
    nc = _get_nc(T, _span)
    wqT, bq, woT, boT = _prep_weights(w_qkv, b_qkv, w_out, b_out)
    bf = ml_dtypes.bfloat16
    in_maps = []
    for b in range(B):
        in_maps.append(
            {
                "x": x[b].astype(bf),
                "wqT": wqT,
                "bqT": bq,
                "woT": woT,
                "boT": boT,
            }
        )
    res = run_bass_kernel_spmd(nc, in_maps, list(range(N_CORES)), trace=_trace)
    out = np.stack([res.results[b]["outT"].T for b in range(B)], axis=0)
    if _trace:
        kernel.last_exec_time_ns = res.exec_time_ns
        kernel.last_results = res
    return out.astype(np.float32)


# revision 16
# speedup vs baseline: 1.2142x; 1.0034x over previous
"""Trainium2 Bass kernel for nn_MultiHeadAttention_53463752900838.

Math (per batch element b, one NeuronCore each — pure data parallel over B=8):
  qkv = w_qkv @ x + b_qkv                     (3072, T)
  q,k,v per head h: (64, T);  q scaled by 1/8 (folded into weights on host)
  scores[t,h,g] = sum_d q[h,d,t] k[g,d,t]     per-timestep 16x16 Gram matrix
  attn = softmax over t  (per (h,g) pair)
  context[h,d,t] = sum_g attn[t,h,g] v[g,d,t]
  out = w_out @ context + b_out               (1024, T)

Kernel layout strategy (all bf16 matmuls, fp32 PSUM accumulation):
  Pass 1 (per 256-t span): project QKV in natural (o, t) orientation,
    marshal per-head blocks into
      QT (64d, (h,t)) / KT (64d, (g,t)) / VT (16g, (d,t))
    via SBUF->SBUF DMA; per-t 16x16 scores matmuls (lhsT=KT slice, rhs=QT
    slice); fused exp during PSUM evac on ScalarE; running Z sums.
    exp(S) and VT spill to DRAM.
  Pass 2 (per span): reload, normalize by 1/Z, per-t context matmuls with
    tile_position column tiling, re-marshal context to channel-major via
    SBUF->SBUF DMA, final projection as out^T (t, o), host transposes back.
"""

import os
import sys
import contextlib

import numpy as np
import ml_dtypes

for p in ("/opt/trn_rl_repo",):
    if p not in sys.path and os.path.isdir(p):
        sys.path.insert(0, p)

import concourse.bass as bass
import concourse.tile as tile
from concourse import mybir
from concourse.bass_utils import run_bass_kernel_spmd

F32 = mybir.dt.float32
BF16 = mybir.dt.bfloat16

N_CORES = 8
C = 1024
H = 16
DK = 64
OC3 = 3072


_WAITS2_OK = {
    "InstMatmult",
    "InstLdweights",
    "InstTensorCopy",
    "InstActivation",
    "InstTensorTensor",
    "InstTensorReduce",
    "InstDMACopy",
    "InstTensorScalarPtr",
    "InstMemset",
}


def _split_sync_waits(nc, limit=1):
    """walrus codegen rejects too many semaphore waits per instruction (CTRL
    class takes 1); hoist overflow waits onto NoOps inserted before the
    offending instruction. Compute/DMA instructions take 2."""
    counter = [0]
    n_split = 0
    for fn in nc.m.functions:
        for bb in fn.blocks:
            out = []
            for ins in bb.instructions:
                si = getattr(ins, "sync_info", None)
                waits = list(si.on_wait) if (si is not None and si.on_wait) else []
                if len(waits) > limit:
                    n_split += 1
                    extra, keep = waits[:-limit], waits[-limit:]
                    for i in range(0, len(extra), limit):
                        counter[0] += 1
                        out.append(
                            mybir.InstNoOp(
                                name=f"I-wsplit-{counter[0]}",
                                opcode="NoOp",
                                engine=ins.engine,
                                ins=[],
                                outs=[],
                                sync_info=mybir.SyncInfo(
                                    on_wait=list(extra[i : i + limit]), on_update=[]
                                ),
                            )
                        )
                    si.on_wait = keep
                out.append(ins)
            bb.instructions[:] = out
    return n_split


def build_kernel(T=4096, SPAN=256):
    NSPAN = T // SPAN
    nc = bass.Bass("TRN2", target_bir_lowering=False, debug=False)

    x_in = nc.dram_tensor("x", [C, T], BF16, kind="ExternalInput").ap()
    wq_in = nc.dram_tensor("wqT", [C, OC3], BF16, kind="ExternalInput").ap()
    bq_in = nc.dram_tensor("bqT", [1, OC3], BF16, kind="ExternalInput").ap()
    wo_in = nc.dram_tensor("woT", [C, C], BF16, kind="ExternalInput").ap()
    bo_in = nc.dram_tensor("boT", [1, C], BF16, kind="ExternalInput").ap()
    out_t = nc.dram_tensor("outT", [T, C], F32, kind="ExternalOutput").ap()
    # DRAM scratch: exp(scores) (g, (h,t)) and VT (g, (d,t)) per span
    se_d = nc.dram_tensor("se_d", [16, H * T], BF16).ap()
    vt_d = nc.dram_tensor("vt_d", [16, DK * T], BF16).ap()

    Exp = mybir.ActivationFunctionType.Exp
    Copy = mybir.ActivationFunctionType.Copy
    ADD = mybir.AluOpType.add
    MUL = mybir.AluOpType.mult

    with tile.TileContext(nc) as tc, contextlib.ExitStack() as octx:
        const = octx.enter_context(tc.tile_pool(name="const", bufs=1))
        wo_sb = []
        for k in range(8):
            w = const.tile([128, C], BF16, tag=f"wo{k}")
            nc.sync.dma_start(w[:], wo_in[k * 128 : (k + 1) * 128, :])
            wo_sb.append(w)
        bo_sb = const.tile([1, C], BF16, tag="bo")
        nc.sync.dma_start(bo_sb[:], bo_in)
        ones_t = const.tile([1, SPAN], BF16, tag="ones_t")
        nc.gpsimd.memset(ones_t[:], 1.0)
        ones128 = const.tile([1, 128], BF16, tag="ones128")
        nc.gpsimd.memset(ones128[:], 1.0)
        zacc = const.tile([16, 16], F32, tag="zacc")
        rrec = const.tile([16, 16], F32, tag="rrec")

        # ---------------- PASS 1 ----------------
        with contextlib.ExitStack() as ctx:
            wpool = ctx.enter_context(tc.tile_pool(name="wq", bufs=1))
            wq_sb = []
            for k in range(8):
                w = wpool.tile([128, OC3], BF16, tag=f"wq{k}")
                nc.sync.dma_start(w[:], wq_in[k * 128 : (k + 1) * 128, :])
                wq_sb.append(w)
            bq_sb = wpool.tile([1, OC3], BF16, tag="bq")
            nc.sync.dma_start(bq_sb[:], bq_in)

            xpool = ctx.enter_context(tc.tile_pool(name="x", bufs=2))
            stpool = ctx.enter_context(tc.tile_pool(name="stage", bufs=2))
            qkpool = ctx.enter_context(tc.tile_pool(name="qkt", bufs=2))
            vtpool = ctx.enter_context(tc.tile_pool(name="vt", bufs=1))
            sepool = ctx.enter_context(tc.tile_pool(name="se", bufs=2))
            zpool = ctx.enter_context(tc.tile_pool(name="zp", bufs=2))
            ps_a = ctx.enter_context(tc.tile_pool(name="psA", bufs=3, space="PSUM"))
            ps_s = ctx.enter_context(tc.tile_pool(name="psS", bufs=3, space="PSUM"))

            for s in range(NSPAN):
                t0 = s * SPAN
                xs = []
                for k in range(8):
                    xk = xpool.tile([128, SPAN], BF16, tag=f"x{k}")
                    nc.sync.dma_start(xk[:], x_in[k * 128 : (k + 1) * 128, t0 : t0 + SPAN])
                    xs.append(xk)

                qt = qkpool.tile([64, H * SPAN], BF16, tag="qt")
                kt = qkpool.tile([64, H * SPAN], BF16, tag="kt")
                vt = vtpool.tile([16, DK * SPAN], BF16, tag="vt")

                stages = {}
                for kind in range(3):  # 0=q, 1=k, 2=v
                    stages[kind] = stpool.tile(
                        [128, 8 * SPAN], BF16, tag=f"st{kind}", name=f"st{kind}"
                    )
                for m in range(24):
                    kind, mm = divmod(m, 8)
                    ps = ps_a.tile([128, SPAN], F32, tag="psA")
                    nc.tensor.matmul(
                        ps[:],
                        lhsT=bq_sb[0:1, m * 128 : (m + 1) * 128],
                        rhs=ones_t[:],
                        start=True,
                        stop=False,
                    )
                    for k in range(8):
                        nc.tensor.matmul(
                            ps[:],
                            lhsT=wq_sb[k][:, m * 128 : (m + 1) * 128],
                            rhs=xs[k][:],
                            start=False,
                            stop=(k == 7),
                        )
                    stg = stages[kind][:, mm * SPAN : (mm + 1) * SPAN]
                    if m % 2 == 0:
                        nc.vector.tensor_copy(stg, ps[:])
                    else:
                        nc.scalar.activation(stg, ps[:], Copy)
                # marshal: Q/K via 2 strided HWDGE copies each; V via 8 SWDGE
                for dst, kind in ((qt, 0), (kt, 1)):
                    src = stages[kind]
                    for par in range(2):
                        nc.sync.dma_start(
                            dst[0:64, :].rearrange(
                                "p (m par t) -> p m par t", m=8, par=2
                            )[:, :, par, :],
                            src[par * 64 : (par + 1) * 64, :]
                            .rearrange("p (m t) -> p m t", m=8),
                        )
                for mm in range(8):
                    nc.gpsimd.dma_start(
                        vt[2 * mm : 2 * mm + 2, :].rearrange("p (d t) -> p d t", d=DK),
                        stages[2][:, mm * SPAN : (mm + 1) * SPAN],
                    )

                qtv = qt[:].rearrange("p (h t) -> p t h", h=H)
                ktv = kt[:].rearrange("p (g t) -> p t g", g=H)
                se = sepool.tile([16, H * SPAN], BF16, tag="se")
                sev = se[:].rearrange("p (h t) -> p t h", h=H)
                for blk in range(SPAN // 32):
                    pss = ps_s.tile([16, 512], F32, tag="psS")
                    for s32 in range(32):
                        tl = blk * 32 + s32
                        nc.tensor.matmul(
                            pss[:, s32 * 16 : (s32 + 1) * 16],
                            lhsT=ktv[:, tl, :],
                            rhs=qtv[:, tl, :],
                            start=True,
                            stop=True,
                        )
                    nc.scalar.activation(
                        sev[:, blk * 32 : (blk + 1) * 32, :],
                        pss[:].rearrange("p (t h) -> p t h", h=H),
                        Exp,
                    )
                zp = zpool.tile([16, 16], F32, tag="zp")
                nc.vector.tensor_reduce(
                    zp[:],
                    se[:].rearrange("p (h t) -> p h t", h=H),
                    axis=mybir.AxisListType.X,
                    op=ADD,
                )
                if s == 0:
                    nc.vector.tensor_copy(zacc[:], zp[:])
                else:
                    nc.vector.tensor_tensor(out=zacc[:], in0=zacc[:], in1=zp[:], op=ADD)
                nc.sync.dma_start(se_d[:, s * H * SPAN : (s + 1) * H * SPAN], se[:])
                nc.sync.dma_start(vt_d[:, s * DK * SPAN : (s + 1) * DK * SPAN], vt[:])

            nc.vector.reciprocal(rrec[:], zacc[:])

        # ---------------- PASS 2 ----------------
        with contextlib.ExitStack() as ctx:
            sepool = ctx.enter_context(tc.tile_pool(name="se2", bufs=2))
            vtpool = ctx.enter_context(tc.tile_pool(name="vt2", bufs=2))
            apool = ctx.enter_context(tc.tile_pool(name="attn", bufs=2))
            cpool = ctx.enter_context(tc.tile_pool(name="csb", bufs=2))
            cnpool = ctx.enter_context(tc.tile_pool(name="cnat", bufs=1))
            opool = ctx.enter_context(tc.tile_pool(name="osb", bufs=2))
            ps_c = ctx.enter_context(tc.tile_pool(name="psC", bufs=4, space="PSUM"))
            ps_o = ctx.enter_context(tc.tile_pool(name="psO", bufs=3, space="PSUM"))

            rbc = rrec[:].unsqueeze(2).broadcast_to([16, 16, SPAN])

            NW = min(4, NSPAN)
            SPC = NW * SPAN  # context accumulation block (1024 t)
            for sb_ in range(NSPAN // NW):
                tB0 = sb_ * SPC
                # C_sb[32j+h, d*256 + w*64 + u]: t_in_block = w*SPAN + j*64 + u
                csb = cpool.tile([128, DK * 64 * NW], BF16, tag="csb")
                for w in range(NW):
                    s = sb_ * NW + w
                    se = sepool.tile([16, H * SPAN], BF16, tag="se2")
                    nc.sync.dma_start(se[:], se_d[:, s * H * SPAN : (s + 1) * H * SPAN])
                    vt = vtpool.tile([16, DK * SPAN], BF16, tag="vt2")
                    nc.sync.dma_start(vt[:], vt_d[:, s * DK * SPAN : (s + 1) * DK * SPAN])

                    at = apool.tile([16, H * SPAN], BF16, tag="attn")
                    nc.vector.tensor_tensor(
                        out=at[:].rearrange("p (h t) -> p h t", h=H),
                        in0=se[:].rearrange("p (h t) -> p h t", h=H),
                        in1=rbc,
                        op=MUL,
                    )

                    atv = at[:].rearrange("p (h t) -> p t h", h=H)
                    vtv = vt[:].rearrange("p (d t) -> p t d", d=DK)
                    for q in range(8):
                        psc = ps_c.tile([128, 512], F32, tag="psC")
                        for j in range(4):
                            for s8 in range(8):
                                tl = j * 64 + q * 8 + s8
                                nc.tensor.matmul(
                                    psc[32 * j : 32 * j + 16, s8 * 64 : (s8 + 1) * 64],
                                    lhsT=atv[:, tl, :],
                                    rhs=vtv[:, tl, :],
                                    start=True,
                                    stop=True,
                                    tile_position=(0, 32 * j),
                                )
                        csb_dst = csb[:].rearrange("p (d tj) -> p tj d", d=DK)[
                            :, w * 64 + q * 8 : w * 64 + (q + 1) * 8, :
                        ]
                        psc_src = psc[:].rearrange("p (s d) -> p s d", s=8)
                        if q % 2 == 0:
                            nc.vector.tensor_copy(csb_dst, psc_src)
                        else:
                            nc.scalar.activation(csb_dst, psc_src, Copy)

                # marshal: Cnat rows (h%2)*64+d, free = k*SPC + w*SPAN + j*64 + u
                cnat = cnpool.tile([128, 8 * SPC], BF16, tag="cnat")
                for j in range(4):
                    for k in range(8):
                        nc.gpsimd.dma_start(
                            cnat[:, :]
                            .rearrange("p (kk w j u) -> p kk w j u", kk=8, w=NW, j=4)[
                                :, k, :, j, :
                            ],
                            csb[32 * j + 2 * k : 32 * j + 2 * k + 2, :].rearrange(
                                "p (d w u) -> p d w u", d=DK, w=NW
                            ),
                        )

                for mt in range(SPC // 128):
                    for n in range(2):
                        pso = ps_o.tile([128, 512], F32, tag="psO")
                        nc.tensor.matmul(
                            pso[:],
                            lhsT=ones128[:],
                            rhs=bo_sb[0:1, n * 512 : (n + 1) * 512],
                            start=True,
                            stop=False,
                        )
                        for k in range(8):
                            nc.tensor.matmul(
                                pso[:],
                                lhsT=cnat[:, k * SPC + mt * 128 : k * SPC + mt * 128 + 128],
                                rhs=wo_sb[k][:, n * 512 : (n + 1) * 512],
                                start=False,
                                stop=(k == 7),
                            )
                        osb = opool.tile([128, 512], F32, tag="osb")
                        nc.scalar.activation(osb[:], pso[:], Copy)
                        nc.sync.dma_start(
                            out_t[tB0 + mt * 128 : tB0 + mt * 128 + 128, n * 512 : (n + 1) * 512],
                            osb[:],
                        )

    _split_sync_waits(nc, limit=1)
    return nc


_NC_CACHE = {}


def _get_nc(T, SPAN):
    key = (T, SPAN)
    if key not in _NC_CACHE:
        _NC_CACHE[key] = build_kernel(T, SPAN)
    return _NC_CACHE[key]


def _prep_weights(w_qkv, b_qkv, w_out, b_out):
    bf = ml_dtypes.bfloat16
    w3 = w_qkv.reshape(H, 192, C).astype(np.float32)
    qw = (w3[:, :DK, :] / 8.0).reshape(H * DK, C)
    kw = w3[:, DK : 2 * DK, :].reshape(H * DK, C)
    vw = w3[:, 2 * DK :, :].reshape(H * DK, C)
    wqT = np.concatenate([qw, kw, vw], axis=0).T.copy().astype(bf)  # (C, 3072)
    b3 = b_qkv.reshape(H, 192).astype(np.float32)
    bq = np.concatenate(
        [(b3[:, :DK] / 8.0).reshape(-1), b3[:, DK : 2 * DK].reshape(-1), b3[:, 2 * DK :].reshape(-1)]
    ).reshape(1, OC3).astype(bf)
    woT = w_out.T.copy().astype(bf)  # (C, C) rows = (h,d) h-major
    boT = b_out.reshape(1, C).astype(bf)
    return wqT, bq, woT, boT


def kernel(x, w_qkv, b_qkv, w_out, b_out, _trace=False, _span=256):
    B, _, T = x.shape
    assert B == N_CORES
    nc = _get_nc(T, _span)
    wqT, bq, woT, boT = _prep_weights(w_qkv, b_qkv, w_out, b_out)
    bf = ml_dtypes.bfloat16
    in_maps = []
    for b in range(B):
        in_maps.append(
            {
                "x": x[b].astype(bf),
                "wqT": wqT,
                "bqT": bq,
                "woT": woT,
                "boT": boT,
            }
        )
    res = run_bass_kernel_spmd(nc, in_maps, list(range(N_CORES)), trace=_trace)
    out = np.stack([res.results[b]["outT"].T for b in range(B)], axis=0)
    if _trace:
        kernel.last_exec_time_ns = res.exec_time_ns
        kernel.last_results = res
    return out.astype(np.float32)
